# revision 45
# baseline (speedup 1.0000x reference)
"""Selective SSM (Mamba-1 style) layer on 8 Trainium2 NeuronCores — v9.

Sharding: core c -> batch b = c // 2, d_model half dh = c % 2 (512 channels).
Cores fully independent (recurrence elementwise in d); no collectives.

The DVE tensor_tensor_scan is the hard bottleneck: 4.33 us per [128,2048]
tile regardless of dtype (2 cyc/elem, no fast modes), 64 tiles = 277 us.
DVE also does the u/prod muls (2x_1p, ~1.07 us) and dtx — ~424 us busy.
Offloading muls to GPSIMD is net-negative: every 2-input DVE op holds the
DVE/GpSimd shared SBUF port, so Pool ops and DVE scans fully serialize
(measured 10.2 us per scan+2-half-mul group vs 5.7 scan-alone). The y
n-reduction stays on PE (identity matmuls into PSUM); exps live on ACT.
Under load the chip throttles (activity limit ~0.56): DVE drops to ~0.8x
clock, so run-to-run wall time varies ~485-580 us; minimize WORK, not
just overlap. v9 vs v5 (517-531 us):
  - one activation table for the whole kernel (natural_log_exp_and_others
    covers exp/ln/identity/copy): insert_act_table_loads is fed a
    narrowed table map and hoists a single load; v5 paid a 1.28 us
    ACT_TABLE_LOAD around every softplus Exp<->Ln pair.
  - all broadcast/store DMAs ride the sync queue; the scalar engine
    issues only startup DMAs, so ACT ops never queue behind ring-full
    dma_start stalls (-18 us).
  - B and C projections merged into one matmul set (wbc [D,2N] -> one
    [2N,SEQ] PSUM region in the startup-idle ps_y banks, one bias ACT).
  - startup is DMA-paced: per-ring HWDGE bandwidth is only ~114 GB/s, so
    (wd[k], xt[k]) pairs alternate across the two rings and the delta-m0
    + B/C matmul loops are k-major, consuming each tile as it lands;
    small consts are host-packed into single DMAs (wbc/abd/dskid);
    a PE-warmup matmul string holds the pstate up while DMAs land.
  - skip term computed by the PE opener matmul (host sends diag(D_skip);
    opener = dskdiag @ xt) instead of a DVE tensor_scalar.
  - y is stored d-major ([DL,SEQ] bf16) and transposed on the host
    during unshard — drops all on-device xbar transposes (-8 MB DMA).
  - delta projections m1..3 pipelined into the half-0 scan loop.
  - last m's half-1 output stored in 4 seq chunks (shorter drain).
Structure retained:
  - n-reduction via PE identity-matmul planes; running y16 opens half 1;
    ACT copies PSUM -> y16 bf16; bar exps on ACT (bf16); softplus Exp
    in-place on PSUM; 32 B/C partition-broadcasts via DRAM bounce in an
    18-slot ring.
"""

import numpy as np
import ml_dtypes
from contextlib import ExitStack

import concourse.bacc as bacc
import concourse.bass as bass
import concourse.mybir as mybir
import concourse.tile as tile
from concourse.bass_utils import run_bass_kernel_spmd

BF16 = ml_dtypes.bfloat16
F32 = mybir.dt.float32
B16 = mybir.dt.bfloat16

B_SZ, SEQ, D, N = 4, 2048, 1024, 16
DL = 512            # d_model channels per core
ND = DL // 128      # 4 d-tiles
NK = D // 128       # 8 contraction tiles
TB = SEQ // 512     # 4 moving-dim blocks for matmul
NHALF = 2
NH = N // NHALF     # 8 states per half

_CACHE = {}

_ACT_TABLE = "natural_log_exp_and_others"


def _patch_act_tables(nc):
    """Narrow the act-table map so insert_act_table_loads hoists ONE load
    of natural_log_exp_and_others (covers exp/ln/identity/copy) instead of
    swapping tables around every Exp<->Ln pair."""
    from concourse import hw_specs
    full = hw_specs.get_activation_tables(nc.m.arch)
    assert _ACT_TABLE in full, list(full)
    patched = {k: (v if k == _ACT_TABLE else set()) for k, v in full.items()}
    bacc.get_activation_tables = lambda arch: patched


def _build():
    if "nc" in _CACHE:
        return _CACHE["nc"]
    mult = mybir.AluOpType.mult
    add = mybir.AluOpType.add

    nc = bacc.Bacc("TRN2", target_bir_lowering=False, debug=False, num_devices=8)

    # x arrives pre-transposed [D, SEQ] and channel-permuted (own 512 first).
    xb16_d = nc.dram_tensor("xb16", [D, SEQ], B16, kind="ExternalInput")
    wd16_d = nc.dram_tensor("wd16", [D, DL], B16, kind="ExternalInput")
    # wbc pre-tiled to SBUF layout [p, (k, 2N)] so it loads as ONE DMA
    wbc_d = nc.dram_tensor("wbc", [128, NK * 2 * N], B16,
                           kind="ExternalInput")
    # packed: col block m = [A[m-tile] | bd[m-tile] | 0], SBUF layout
    abd_d = nc.dram_tensor("abd", [128, ND * (N + 2)], F32,
                           kind="ExternalInput")
    bbc_d = nc.dram_tensor("bbc", [2 * N, 1], F32, kind="ExternalInput")
    # packed: [diag(D_skip) m-tiles | identity]
    dskid_d = nc.dram_tensor("dskid", [128, (ND + 1) * 128], B16,
                             kind="ExternalInput")
    # y stays d-major on device; the host transposes during unshard.
    y_d = nc.dram_tensor("y", [DL, SEQ], B16, kind="ExternalOutput")

    with tile.TileContext(nc) as tc, ExitStack() as ctx:
        consts = ctx.enter_context(tc.tile_pool(name="consts", bufs=1))
        persist = ctx.enter_context(tc.tile_pool(name="persist", bufs=1))
        ps_mm = ctx.enter_context(tc.tile_pool(name="ps_mm", bufs=2, space="PSUM"))
        ps_y = ctx.enter_context(tc.tile_pool(name="ps_y", bufs=1, space="PSUM"))
        xpool = ctx.enter_context(tc.tile_pool(name="xpool", bufs=8))
        bcast = ctx.enter_context(tc.tile_pool(name="bcast", bufs=18))
        work = ctx.enter_context(tc.tile_pool(name="work", bufs=2))
        dram = ctx.enter_context(tc.tile_pool(name="dram", bufs=1, space="DRAM"))

        # ---- DMA schedule: tiny consts first, then (wd[k], xt[k]) pairs
        # alternating across the two rings so the projections' k-major
        # matmul loops consume tiles as they land (~114 GB/s per ring).
        wball = consts.tile([128, NK * 2 * N], B16, tag="wball", name="wball")
        nc.sync.dma_start(wball[:], wbc_d[:, :])
        wbc_sb = [wball[:, k * 2 * N:(k + 1) * 2 * N] for k in range(NK)]
        abdall = consts.tile([128, ND * (N + 2)], F32, tag="abdall",
                             name="abdall")
        nc.scalar.dma_start(abdall[:], abd_d[:, :])
        abd = [abdall[:, m * (N + 2):(m + 1) * (N + 2)] for m in range(ND)]
        a_sb = [t[:, 0:N] for t in abd]
        bd_sb = [t[:, N:N + 1] for t in abd]
        bbc = consts.tile([2 * N, 1], F32, tag="bbc", name="bbc")
        nc.scalar.dma_start(bbc[:], bbc_d[:, :])
        dskid = consts.tile([128, (ND + 1) * 128], B16, tag="dskid",
                            name="dskid")
        nc.scalar.dma_start(dskid[:], dskid_d[:, :])
        dsk_sb = [dskid[:, m * 128:(m + 1) * 128] for m in range(ND)]
        id16_sb = dskid[:, ND * 128:(ND + 1) * 128]

        wdall = consts.tile([128, NK * DL], B16, tag="wdall", name="wdall")
        wd_sb = [wdall[:, k * DL:(k + 1) * DL] for k in range(NK)]
        xt = []
        for k in range(NK):
            t = xpool.tile([128, SEQ], B16, tag="xt", name=f"xt{k}")
            xt.append(t)
        for k in range(NK):
            eng = nc.sync if k % 2 == 0 else nc.scalar
            eng.dma_start(wdall[:, k * DL:(k + 1) * DL],
                          wd16_d[k * 128:(k + 1) * 128, :])
            if k in (4, 5):
                # third parallel DMA stream via the gpsimd SWDGE queue
                # (DVE is idle at startup, so no shared-port contention)
                nc.gpsimd.dma_start(xt[k][:], xb16_d[k * 128:(k + 1) * 128, :])
            else:
                eng.dma_start(xt[k][:], xb16_d[k * 128:(k + 1) * 128, :])

        # ---- persist tiles for the scan ----
        dt16 = [persist.tile([128, SEQ], B16, tag=f"dt{m}", name=f"dtv{m}")
                for m in range(ND)]
        dtx = [persist.tile([128, SEQ], B16, tag=f"dtx{m}", name=f"dtx{m}")
               for m in range(ND)]
        # yd16[m]: holds dskx until the half-0 opener consumed it, then the
        # running/final y16.
        yd16 = [persist.tile([128, SEQ], B16, tag=f"yd{m}", name=f"yd{m}")
                for m in range(ND)]

        def delta_proj_pe_act(m):
            pss = []
            for th in range(2):
                ps = ps_mm.tile([128, 1024], F32, tag="mm", name="mmps")
                for sb in range(2):
                    for k in range(NK):
                        nc.tensor.matmul(
                            ps[:, sb * 512:(sb + 1) * 512],
                            wd_sb[k][:, m * 128:(m + 1) * 128],
                            xt[k][:, th * 1024 + sb * 512:th * 1024 + (sb + 1) * 512],
                            start=(k == 0), stop=(k == NK - 1),
                        )
                nc.scalar.activation(
                    ps[:], ps[:], mybir.ActivationFunctionType.Exp,
                    bias=bd_sb[m], scale=1.0,
                )
                pss.append(ps)
            for th in range(2):
                nc.scalar.activation(
                    dt16[m][:, th * 1024:(th + 1) * 1024], pss[th][:],
                    mybir.ActivationFunctionType.Ln, bias=1.0, scale=1.0,
                )

        def delta_proj_dve(m):
            nc.vector.tensor_mul(dtx[m][:], dt16[m][:], xt[m][:])

        def emit_plane(half, m, j, yps):
            n = half * NH + j
            bar = work.tile([128, SEQ], B16, tag="bar", name="barv",
                            bufs=3)
            nc.scalar.activation(
                bar[:], dt16[m][:],
                mybir.ActivationFunctionType.Exp,
                bias=0.0, scale=a_sb[m][:, n:n + 1],
            )
            u = work.tile([128, SEQ], B16, tag="u", name="uv")
            nc.vector.tensor_mul(u[:], dtx[m][:], breps[n][:])
            h = work.tile([128, SEQ], B16, tag="h", name="hv")
            nc.vector.tensor_tensor_scan(
                h[:], bar[:], u[:], 0.0, op0=mult, op1=add,
            )
            prod = work.tile([128, SEQ], B16, tag="prod", name="prodv",
                             bufs=3)
            nc.vector.tensor_mul(prod[:], h[:], creps[n][:])
            last = (j == NH - 1)
            for tb in range(TB):
                nc.tensor.matmul(
                    yps[:, tb * 512:(tb + 1) * 512], id16_sb[:],
                    prod[:, tb * 512:(tb + 1) * 512],
                    start=False, stop=last,
                )

        # ---- startup projections, k-major: delta(m0) + B/C matmuls for
        # tile k are emitted together so PE consumes each (wd[k], xt[k])
        # pair as its DMA lands; both projections finish ~1us after xt[7].
        # B/C accumulates in the (startup-idle) ps_y banks; delta m0 in
        # ps_mm. A short PE warmup ramps the pstate while DMAs arrive.
        warm = work.tile([128, 512], B16, tag="warm", name="warm", bufs=1)
        nc.gpsimd.memset(warm[:], 0.0)
        wps = ps_mm.tile([128, 1024], F32, tag="mm", name="warmps")
        for _ in range(40):
            nc.tensor.matmul(wps[:, 0:512], warm[:, 0:128], warm[:],
                             start=True, stop=True)

        pss0 = [ps_mm.tile([128, 1024], F32, tag="mm", name=f"mmps0{th}")
                for th in range(2)]
        psbc = ps_y.tile([128, SEQ], F32, tag="yps", name="psbc")
        for k in range(NK):
            for th in range(2):
                for sb in range(2):
                    nc.tensor.matmul(
                        pss0[th][:, sb * 512:(sb + 1) * 512],
                        wd_sb[k][:, 0:128],
                        xt[k][:, th * 1024 + sb * 512:th * 1024 + (sb + 1) * 512],
                        start=(k == 0), stop=(k == NK - 1),
                    )
            for tb in range(TB):
                nc.tensor.matmul(
                    psbc[0:2 * N, tb * 512:(tb + 1) * 512], wbc_sb[k],
                    xt[k][:, tb * 512:(tb + 1) * 512],
                    start=(k == 0), stop=(k == NK - 1),
                )
        for th in range(2):
            nc.scalar.activation(
                pss0[th][:], pss0[th][:], mybir.ActivationFunctionType.Exp,
                bias=bd_sb[0], scale=1.0,
            )
        for th in range(2):
            nc.scalar.activation(
                dt16[0][:, th * 1024:(th + 1) * 1024], pss0[th][:],
                mybir.ActivationFunctionType.Ln, bias=1.0, scale=1.0,
            )
        bcmat = bcast.tile([128, SEQ], B16, tag="bc16", name="bcmat")[0:2 * N, :]
        nc.scalar.activation(
            bcmat[:], psbc[0:2 * N, :],
            mybir.ActivationFunctionType.Identity, bias=bbc[:], scale=1.0,
        )

        delta_proj_dve(0)

        # bounce B/C to DRAM, then issue ALL 32 broadcasts upfront on the
        # sync queue (keeps the scalar/ACT sequencer free of ring stalls).
        bcmat_dr = dram.tile([2 * N, SEQ], B16, tag="bcmat_dr", name="bcmat_dr")
        nc.sync.dma_start(bcmat_dr[:], bcmat[:])
        breps = []
        creps = []
        for n in range(N):
            br = bcast.tile([128, SEQ], B16, tag="bc16", name="brep")
            nc.sync.dma_start(br[:], bcmat_dr[n:n + 1, :].partition_broadcast(128))
            breps.append(br)
            cr = bcast.tile([128, SEQ], B16, tag="bc16", name="crep")
            nc.sync.dma_start(
                cr[:], bcmat_dr[N + n:N + n + 1, :].partition_broadcast(128))
            creps.append(cr)

        # ---- scan phase; half-0 m-loop carries delta projections 1..3 ----
        for half in range(NHALF):
            for m in range(ND):
                yps = ps_y.tile([128, SEQ], F32, tag="yps", name="ypsv")
                opener = dsk_sb[m] if half == 0 else id16_sb[:]
                src = xt[m] if half == 0 else yd16[m]
                for tb in range(TB):
                    nc.tensor.matmul(
                        yps[:, tb * 512:(tb + 1) * 512], opener,
                        src[:, tb * 512:(tb + 1) * 512],
                        start=True, stop=False,
                    )
                if half == 0 and m == 0:
                    # plane (0,0,0) first so its bar/u/scan aren't queued
                    # behind proj(1)'s ACT ops.
                    emit_plane(0, 0, 0, yps)
                    delta_proj_pe_act(1)
                    delta_proj_dve(1)
                    jrange = range(1, NH)
                else:
                    if half == 0 and m + 1 < ND:
                        delta_proj_pe_act(m + 1)
                        delta_proj_dve(m + 1)
                    jrange = range(NH)
                for j in jrange:
                    emit_plane(half, m, j, yps)
                if half == 0:
                    nc.scalar.activation(
                        yd16[m][:], yps[:],
                        mybir.ActivationFunctionType.Copy, bias=0.0, scale=1.0,
                    )
                else:
                    nsh = 4 if m == ND - 1 else 2
                    shw = SEQ // nsh
                    for sh in range(nsh):
                        hs = slice(sh * shw, (sh + 1) * shw)
                        nc.scalar.activation(
                            yd16[m][:, hs], yps[:, hs],
                            mybir.ActivationFunctionType.Copy,
                            bias=0.0, scale=1.0,
                        )
                        nc.sync.dma_start(
                            y_d[m * 128:(m + 1) * 128, hs], yd16[m][:, hs])

    _patch_act_tables(nc)
    nc.compile()
    _CACHE["nc"] = nc
    return nc


def _in_maps(x, A_log, D_skip, Wd, bd, Wb, bb, Wc, bc):
    A = (-np.exp(np.asarray(A_log, np.float64))).astype(np.float32)
    x = np.asarray(x, np.float32)
    maps = []
    for c in range(8):
        b, dh = c // 2, c % 2
        dsl = slice(dh * DL, (dh + 1) * DL)
        osl = slice((1 - dh) * DL, (2 - dh) * DL)
        perm = np.r_[np.arange(dsl.start, dsl.stop),
                     np.arange(osl.start, osl.stop)]
        wbc = np.concatenate([np.asarray(Wb), np.asarray(Wc)], axis=1)[perm]
        # [p, (k, 2N)] = wbc[k*128+p, :]
        wbcp = (wbc.astype(BF16).reshape(NK, 128, 2 * N)
                .transpose(1, 0, 2).reshape(128, NK * 2 * N))
        bbcm = np.concatenate([np.asarray(bb, np.float32),
                               np.asarray(bc, np.float32)])[:, None]
        Ad = A[dsl]
        bdd = np.asarray(bd, np.float32)[dsl]
        abdp = np.zeros((128, ND * (N + 2)), np.float32)
        for m in range(ND):
            abdp[:, m * (N + 2):m * (N + 2) + N] = Ad[m * 128:(m + 1) * 128]
            abdp[:, m * (N + 2) + N] = bdd[m * 128:(m + 1) * 128]
        dskid = np.zeros((128, (ND + 1) * 128), dtype=BF16)
        dsk = np.asarray(D_skip, np.float32)[dsl]
        for m in range(ND):
            dskid[:, m * 128:(m + 1) * 128][np.arange(128), np.arange(128)] = \
                dsk[m * 128:(m + 1) * 128].astype(BF16)
        dskid[:, ND * 128:(ND + 1) * 128] = np.eye(128, dtype=BF16)
        maps.append({
            "xb16": np.ascontiguousarray(x[b][:, perm].astype(BF16).T),
            "wd16": np.asarray(Wd)[perm][:, dsl].astype(BF16),
            "wbc": np.ascontiguousarray(wbcp),
            "abd": abdp,
            "bbc": np.ascontiguousarray(bbcm),
            "dskid": dskid,
        })
    return maps


def kernel(x, A_log, D_skip, Wd, bd, Wb, bb, Wc, bc, _trace=False):
    nc = _build()
    maps = _in_maps(x, A_log, D_skip, Wd, bd, Wb, bb, Wc, bc)
    res = run_bass_kernel_spmd(nc, maps, list(range(8)), trace=_trace)
    y = np.zeros((B_SZ, SEQ, D), np.float32)
    for c, om in enumerate(res.results):
        b, dh = c // 2, c % 2
        y[b][:, dh * DL:(dh + 1) * DL] = om["y"].astype(np.float32).T
    if _trace:
        kernel.last_result = res
    return y



# revision 53
# speedup vs baseline: 1.2049x; 1.2049x over previous
"""Selective SSM (Mamba-1 style) layer on 8 Trainium2 NeuronCores — v9.

Sharding: core c -> batch b = c // 2, d_model half dh = c % 2 (512 channels).
Cores fully independent (recurrence elementwise in d); no collectives.

The DVE tensor_tensor_scan is the hard bottleneck: 4.33 us per [128,2048]
tile regardless of dtype (2 cyc/elem, no fast modes), 64 tiles = 277 us.
DVE also does the u/prod muls (2x_1p, ~1.07 us) and dtx — ~424 us busy.
Offloading muls to GPSIMD is net-negative: every 2-input DVE op holds the
DVE/GpSimd shared SBUF port, so Pool ops and DVE scans fully serialize
(measured 10.2 us per scan+2-half-mul group vs 5.7 scan-alone). The y
n-reduction stays on PE (identity matmuls into PSUM); exps live on ACT.
Under load the chip throttles (activity limit ~0.56): DVE drops to ~0.8x
clock, so run-to-run wall time varies ~485-580 us; minimize WORK, not
just overlap. v9 vs v5 (517-531 us):
  - one activation table for the whole kernel (natural_log_exp_and_others
    covers exp/ln/identity/copy): insert_act_table_loads is fed a
    narrowed table map and hoists a single load; v5 paid a 1.28 us
    ACT_TABLE_LOAD around every softplus Exp<->Ln pair.
  - all broadcast/store DMAs ride the sync queue; the scalar engine
    issues only startup DMAs, so ACT ops never queue behind ring-full
    dma_start stalls (-18 us).
  - B and C projections merged into one matmul set (wbc [D,2N] -> one
    [2N,SEQ] PSUM region in the startup-idle ps_y banks, one bias ACT).
  - startup is DMA-paced: per-ring HWDGE bandwidth is only ~114 GB/s, so
    (wd[k], xt[k]) pairs alternate across the two rings and the delta-m0
    + B/C matmul loops are k-major, consuming each tile as it lands;
    small consts are host-packed into single DMAs (wbc/abd/dskid);
    a PE-warmup matmul string holds the pstate up while DMAs land.
  - skip term computed by the PE opener matmul (host sends diag(D_skip);
    opener = dskdiag @ xt) instead of a DVE tensor_scalar.
  - y is stored d-major ([DL,SEQ] bf16) and transposed on the host
    during unshard — drops all on-device xbar transposes (-8 MB DMA).
  - delta projections m1..3 pipelined into the half-0 scan loop.
  - last m's half-1 output stored in 4 seq chunks (shorter drain).
Structure retained:
  - n-reduction via PE identity-matmul planes; running y16 opens half 1;
    ACT copies PSUM -> y16 bf16; bar exps on ACT (bf16); softplus Exp
    in-place on PSUM; 32 B/C partition-broadcasts via DRAM bounce in an
    18-slot ring.
"""

import numpy as np
import ml_dtypes
from contextlib import ExitStack

import concourse.bacc as bacc
import concourse.bass as bass
import concourse.mybir as mybir
import concourse.tile as tile
from concourse.bass_utils import run_bass_kernel_spmd

BF16 = ml_dtypes.bfloat16
F32 = mybir.dt.float32
B16 = mybir.dt.bfloat16

B_SZ, SEQ, D, N = 4, 2048, 1024, 16
DL = 512            # d_model channels per core
ND = DL // 128      # 4 d-tiles
NK = D // 128       # 8 contraction tiles
TB = SEQ // 512     # 4 moving-dim blocks for matmul
NHALF = 2
NH = N // NHALF     # 8 states per half

_CACHE = {}

_ACT_TABLE = "natural_log_exp_and_others"


def _patch_act_tables(nc):
    """Narrow the act-table map so insert_act_table_loads hoists ONE load
    of natural_log_exp_and_others (covers exp/ln/identity/copy) instead of
    swapping tables around every Exp<->Ln pair."""
    from concourse import hw_specs
    full = hw_specs.get_activation_tables(nc.m.arch)
    assert _ACT_TABLE in full, list(full)
    patched = {k: (v if k == _ACT_TABLE else set()) for k, v in full.items()}
    bacc.get_activation_tables = lambda arch: patched


def _build():
    if "nc" in _CACHE:
        return _CACHE["nc"]
    mult = mybir.AluOpType.mult
    add = mybir.AluOpType.add

    nc = bacc.Bacc("TRN2", target_bir_lowering=False, debug=False, num_devices=8)

    # x arrives pre-transposed [D, SEQ] and channel-permuted (own 512 first).
    xb16_d = nc.dram_tensor("xb16", [D, SEQ], B16, kind="ExternalInput")
    wd16_d = nc.dram_tensor("wd16", [D, DL], B16, kind="ExternalInput")
    # wbc pre-tiled to SBUF layout [p, (k, 2N)] so it loads as ONE DMA
    wbc_d = nc.dram_tensor("wbc", [128, NK * 2 * N], B16,
                           kind="ExternalInput")
    # packed: col block m = [A[m-tile] | bd[m-tile] | 0], SBUF layout
    abd_d = nc.dram_tensor("abd", [128, ND * (N + 2)], F32,
                           kind="ExternalInput")
    bbc_d = nc.dram_tensor("bbc", [2 * N, 1], F32, kind="ExternalInput")
    # packed: [diag(D_skip) m-tiles | identity]
    dskid_d = nc.dram_tensor("dskid", [128, (ND + 1) * 128], B16,
                             kind="ExternalInput")
    # y stays d-major on device; the host transposes during unshard.
    y_d = nc.dram_tensor("y", [DL, SEQ], B16, kind="ExternalOutput")

    with tile.TileContext(nc) as tc, ExitStack() as ctx:
        consts = ctx.enter_context(tc.tile_pool(name="consts", bufs=1))
        persist = ctx.enter_context(tc.tile_pool(name="persist", bufs=1))
        ps_mm = ctx.enter_context(tc.tile_pool(name="ps_mm", bufs=2, space="PSUM"))
        ps_y = ctx.enter_context(tc.tile_pool(name="ps_y", bufs=1, space="PSUM"))
        xpool = ctx.enter_context(tc.tile_pool(name="xpool", bufs=8))
        bcast = ctx.enter_context(tc.tile_pool(name="bcast", bufs=18))
        work = ctx.enter_context(tc.tile_pool(name="work", bufs=2))
        dram = ctx.enter_context(tc.tile_pool(name="dram", bufs=1, space="DRAM"))

        # ---- DMA schedule: tiny consts first, then (wd[k], xt[k]) pairs
        # alternating across the two rings so the projections' k-major
        # matmul loops consume tiles as they land (~114 GB/s per ring).
        wball = consts.tile([128, NK * 2 * N], B16, tag="wball", name="wball")
        nc.sync.dma_start(wball[:], wbc_d[:, :])
        wbc_sb = [wball[:, k * 2 * N:(k + 1) * 2 * N] for k in range(NK)]
        abdall = consts.tile([128, ND * (N + 2)], F32, tag="abdall",
                             name="abdall")
        nc.scalar.dma_start(abdall[:], abd_d[:, :])
        abd = [abdall[:, m * (N + 2):(m + 1) * (N + 2)] for m in range(ND)]
        a_sb = [t[:, 0:N] for t in abd]
        bd_sb = [t[:, N:N + 1] for t in abd]
        bbc = consts.tile([2 * N, 1], F32, tag="bbc", name="bbc")
        nc.scalar.dma_start(bbc[:], bbc_d[:, :])
        dskid = consts.tile([128, (ND + 1) * 128], B16, tag="dskid",
                            name="dskid")
        nc.scalar.dma_start(dskid[:], dskid_d[:, :])
        dsk_sb = [dskid[:, m * 128:(m + 1) * 128] for m in range(ND)]
        id16_sb = dskid[:, ND * 128:(ND + 1) * 128]

        wdall = consts.tile([128, NK * DL], B16, tag="wdall", name="wdall")
        wd_sb = [wdall[:, k * DL:(k + 1) * DL] for k in range(NK)]
        xt = []
        for k in range(NK):
            t = xpool.tile([128, SEQ], B16, tag="xt", name=f"xt{k}")
            xt.append(t)
        for k in range(NK):
            # xt first: the B/C projection (which gates brep0 -> u000)
            # needs only xt[k]; wd[k] rides behind it for the delta proj.
            eng = nc.sync if k % 2 == 0 else nc.scalar
            eng.dma_start(xt[k][:], xb16_d[k * 128:(k + 1) * 128, :])
            eng.dma_start(wdall[:, k * DL:(k + 1) * DL],
                          wd16_d[k * 128:(k + 1) * 128, :])

        # ---- persist tiles for the scan ----
        dt16 = [persist.tile([128, SEQ], B16, tag=f"dt{m}", name=f"dtv{m}")
                for m in range(ND)]
        dtx = [persist.tile([128, SEQ], B16, tag=f"dtx{m}", name=f"dtx{m}")
               for m in range(ND)]
        # yd16[m]: holds dskx until the half-0 opener consumed it, then the
        # running/final y16.
        yd16 = [persist.tile([128, SEQ], B16, tag=f"yd{m}", name=f"yd{m}")
                for m in range(ND)]

        def delta_proj_pe_act(m):
            pss = []
            for th in range(2):
                ps = ps_mm.tile([128, 1024], F32, tag="mm", name="mmps")
                for sb in range(2):
                    for k in range(NK):
                        nc.tensor.matmul(
                            ps[:, sb * 512:(sb + 1) * 512],
                            wd_sb[k][:, m * 128:(m + 1) * 128],
                            xt[k][:, th * 1024 + sb * 512:th * 1024 + (sb + 1) * 512],
                            start=(k == 0), stop=(k == NK - 1),
                        )
                nc.scalar.activation(
                    ps[:], ps[:], mybir.ActivationFunctionType.Exp,
                    bias=bd_sb[m], scale=1.0,
                )
                pss.append(ps)
            for th in range(2):
                nc.scalar.activation(
                    dt16[m][:, th * 1024:(th + 1) * 1024], pss[th][:],
                    mybir.ActivationFunctionType.Ln, bias=1.0, scale=1.0,
                )

        def delta_proj_dve(m):
            nc.vector.tensor_mul(dtx[m][:], dt16[m][:], xt[m][:])

        def emit_plane(half, m, j, yps):
            n = half * NH + j
            bar = work.tile([128, SEQ], B16, tag="bar", name="barv",
                            bufs=3)
            nc.scalar.activation(
                bar[:], dt16[m][:],
                mybir.ActivationFunctionType.Exp,
                bias=0.0, scale=a_sb[m][:, n:n + 1],
            )
            u = work.tile([128, SEQ], B16, tag="u", name="uv")
            nc.vector.tensor_mul(u[:], dtx[m][:], breps[n][:])
            h = work.tile([128, SEQ], B16, tag="h", name="hv")
            nc.vector.tensor_tensor_scan(
                h[:], bar[:], u[:], 0.0, op0=mult, op1=add,
            )
            prod = work.tile([128, SEQ], B16, tag="prod", name="prodv",
                             bufs=4)
            nc.vector.tensor_mul(prod[:], h[:], creps[n][:])
            last = (j == NH - 1)
            for tb in range(TB):
                nc.tensor.matmul(
                    yps[:, tb * 512:(tb + 1) * 512], id16_sb[:],
                    prod[:, tb * 512:(tb + 1) * 512],
                    start=False, stop=last,
                )

        # ---- startup projections, k-major: delta(m0) + B/C matmuls for
        # tile k are emitted together so PE consumes each (wd[k], xt[k])
        # pair as its DMA lands; both projections finish ~1us after xt[7].
        # B/C accumulates in the (startup-idle) ps_y banks; delta m0 in
        # ps_mm. A short PE warmup ramps the pstate while DMAs arrive.
        warm = work.tile([128, 512], B16, tag="warm", name="warm", bufs=1)
        nc.gpsimd.memset(warm[:], 0.0)
        wps = ps_mm.tile([128, 1024], F32, tag="mm", name="warmps")
        for _ in range(10):
            nc.tensor.matmul(wps[:, 0:512], warm[:, 0:128], warm[:],
                             start=True, stop=True)

        pss0 = [ps_mm.tile([128, 1024], F32, tag="mm", name=f"mmps0{th}")
                for th in range(2)]
        psbc = ps_y.tile([128, SEQ], F32, tag="yps", name="psbc")
        for k in range(NK):
            for th in range(2):
                for sb in range(2):
                    nc.tensor.matmul(
                        pss0[th][:, sb * 512:(sb + 1) * 512],
                        wd_sb[k][:, 0:128],
                        xt[k][:, th * 1024 + sb * 512:th * 1024 + (sb + 1) * 512],
                        start=(k == 0), stop=(k == NK - 1),
                    )
            for tb in range(TB):
                nc.tensor.matmul(
                    psbc[0:2 * N, tb * 512:(tb + 1) * 512], wbc_sb[k],
                    xt[k][:, tb * 512:(tb + 1) * 512],
                    start=(k == 0), stop=(k == NK - 1),
                )
        for th in range(2):
            nc.scalar.activation(
                pss0[th][:], pss0[th][:], mybir.ActivationFunctionType.Exp,
                bias=bd_sb[0], scale=1.0,
            )
        # bias BEFORE the Lns: the bias->bounce->brep0->u000 chain gates
        # the first scan (bar000 has ~6us of slack behind it).
        bcmat = bcast.tile([128, SEQ], B16, tag="bc16", name="bcmat")[0:2 * N, :]
        nc.scalar.activation(
            bcmat[:], psbc[0:2 * N, :],
            mybir.ActivationFunctionType.Identity, bias=bbc[:], scale=1.0,
        )
        for th in range(2):
            nc.scalar.activation(
                dt16[0][:, th * 1024:(th + 1) * 1024], pss0[th][:],
                mybir.ActivationFunctionType.Ln, bias=1.0, scale=1.0,
            )

        delta_proj_dve(0)

        # bounce B/C to DRAM, then issue ALL 32 broadcasts upfront on the
        # sync queue (keeps the scalar/ACT sequencer free of ring stalls).
        bcmat_dr = dram.tile([2 * N, SEQ], B16, tag="bcmat_dr", name="bcmat_dr")
        nc.sync.dma_start(bcmat_dr[:], bcmat[:])
        breps = []
        creps = []
        for n in range(N):
            br = bcast.tile([128, SEQ], B16, tag="bc16", name="brep")
            nc.sync.dma_start(br[:], bcmat_dr[n:n + 1, :].partition_broadcast(128))
            breps.append(br)
            cr = bcast.tile([128, SEQ], B16, tag="bc16", name="crep")
            nc.sync.dma_start(
                cr[:], bcmat_dr[N + n:N + n + 1, :].partition_broadcast(128))
            creps.append(cr)

        # ---- scan phase; half-0 m-loop carries delta projections 1..3 ----
        for half in range(NHALF):
            for m in range(ND):
                yps = ps_y.tile([128, SEQ], F32, tag="yps", name="ypsv")
                opener = dsk_sb[m] if half == 0 else id16_sb[:]
                src = xt[m] if half == 0 else yd16[m]
                for tb in range(TB):
                    nc.tensor.matmul(
                        yps[:, tb * 512:(tb + 1) * 512], opener,
                        src[:, tb * 512:(tb + 1) * 512],
                        start=True, stop=False,
                    )
                if half == 0 and m == 0:
                    # plane (0,0,0) first so its bar/u/scan aren't queued
                    # behind proj(1)'s ACT ops.
                    emit_plane(0, 0, 0, yps)
                    delta_proj_pe_act(1)
                    delta_proj_dve(1)
                    jrange = range(1, NH)
                else:
                    if half == 0 and m + 1 < ND:
                        delta_proj_pe_act(m + 1)
                        delta_proj_dve(m + 1)
                    jrange = range(NH)
                for j in jrange:
                    emit_plane(half, m, j, yps)
                if half == 0:
                    nc.scalar.activation(
                        yd16[m][:], yps[:],
                        mybir.ActivationFunctionType.Copy, bias=0.0, scale=1.0,
                    )
                else:
                    nsh = 4 if m == ND - 1 else 2
                    shw = SEQ // nsh
                    for sh in range(nsh):
                        hs = slice(sh * shw, (sh + 1) * shw)
                        nc.scalar.activation(
                            yd16[m][:, hs], yps[:, hs],
                            mybir.ActivationFunctionType.Copy,
                            bias=0.0, scale=1.0,
                        )
                        nc.sync.dma_start(
                            y_d[m * 128:(m + 1) * 128, hs], yd16[m][:, hs])

    _patch_act_tables(nc)
    nc.compile()
    _CACHE["nc"] = nc
    return nc


def _in_maps(x, A_log, D_skip, Wd, bd, Wb, bb, Wc, bc):
    A = (-np.exp(np.asarray(A_log, np.float64))).astype(np.float32)
    x = np.asarray(x, np.float32)
    maps = []
    for c in range(8):
        b, dh = c // 2, c % 2
        dsl = slice(dh * DL, (dh + 1) * DL)
        osl = slice((1 - dh) * DL, (2 - dh) * DL)
        perm = np.r_[np.arange(dsl.start, dsl.stop),
                     np.arange(osl.start, osl.stop)]
        wbc = np.concatenate([np.asarray(Wb), np.asarray(Wc)], axis=1)[perm]
        # [p, (k, 2N)] = wbc[k*128+p, :]
        wbcp = (wbc.astype(BF16).reshape(NK, 128, 2 * N)
                .transpose(1, 0, 2).reshape(128, NK * 2 * N))
        bbcm = np.concatenate([np.asarray(bb, np.float32),
                               np.asarray(bc, np.float32)])[:, None]
        Ad = A[dsl]
        bdd = np.asarray(bd, np.float32)[dsl]
        abdp = np.zeros((128, ND * (N + 2)), np.float32)
        for m in range(ND):
            abdp[:, m * (N + 2):m * (N + 2) + N] = Ad[m * 128:(m + 1) * 128]
            abdp[:, m * (N + 2) + N] = bdd[m * 128:(m + 1) * 128]
        dskid = np.zeros((128, (ND + 1) * 128), dtype=BF16)
        dsk = np.asarray(D_skip, np.float32)[dsl]
        for m in range(ND):
            dskid[:, m * 128:(m + 1) * 128][np.arange(128), np.arange(128)] = \
                dsk[m * 128:(m + 1) * 128].astype(BF16)
        dskid[:, ND * 128:(ND + 1) * 128] = np.eye(128, dtype=BF16)
        maps.append({
            "xb16": np.ascontiguousarray(x[b][:, perm].astype(BF16).T),
            "wd16": np.asarray(Wd)[perm][:, dsl].astype(BF16),
            "wbc": np.ascontiguousarray(wbcp),
            "abd": abdp,
            "bbc": np.ascontiguousarray(bbcm),
            "dskid": dskid,
        })
    return maps


def kernel(x, A_log, D_skip, Wd, bd, Wb, bb, Wc, bc, _trace=False):
    nc = _build()
    maps = _in_maps(x, A_log, D_skip, Wd, bd, Wb, bb, Wc, bc)
    res = run_bass_kernel_spmd(nc, maps, list(range(8)), trace=_trace)
    y = np.zeros((B_SZ, SEQ, D), np.float32)
    for c, om in enumerate(res.results):
        b, dh = c // 2, c % 2
        y[b][:, dh * DL:(dh + 1) * DL] = om["y"].astype(np.float32).T
    if _trace:
        kernel.last_result = res
    return y

